# revision 1
# baseline (speedup 1.0000x reference)
"""Deformable multi-head sparse attention (DMSA) Bass kernel for Trainium2.

Contract: kernel(**inputs) takes the FULL unsharded inputs (as produced by
setup_inputs()) and returns the FULL output (B, 384, 56, 56) float32.
Internally shards batch B=8 across 8 NeuronCores (pure data parallel,
no collectives), one batch element per core.

Self-contained: hardcodes all shapes; does not read any sibling files.
"""
import sys

for _p in ("/opt/trn_rl_repo", "/opt/pypackages"):
    if _p not in sys.path:
        sys.path.insert(0, _p)

import numpy as np

import concourse.bass as bass
import concourse.mybir as mybir
import concourse.tile as tile
from concourse import bacc
from concourse import bass_utils

F32 = mybir.dt.float32
F32R = mybir.dt.float32r
I16 = mybir.dt.int16
I32 = mybir.dt.int32
AF = mybir.ActivationFunctionType
OP = mybir.AluOpType

# problem constants
B = 8
DIM = 384
DIM_HEAD = 64
NUM_HEAD = 6
G = 3            # deformable groups
NGD = 128        # channels per group
H = 56
W = 56
HW = H * W       # 3136
HO = 28
WO = 28
L = HO * WO      # 784
SCALE = DIM_HEAD ** -0.5
BN_EPS = 1e-6
A = (W - 1) / WO   # 55/28, same for y since H==W and HO==WO
PADD = 60          # padded dwconv input edge (56 + 2*2)

QC = 448           # q-position chunk (free dim of attention matmuls)
NQC = HW // QC     # 7
LC = 112           # kv-position chunk (partition dim of S^T)
NLC = L // LC      # 7


def _r(ap):
    return ap


def build_nc(gelu_exact: bool = True, stop_after: str = ""):
    """Build the per-core Bass program (SPMD: same NEFF on all 8 cores)."""
    nc = bacc.Bacc("TRN2", target_bir_lowering=False, debug=False, num_devices=B)

    din = {}
    def dt_in(name, shape, dtype=F32):
        din[name] = nc.dram_tensor(name, shape, dtype, kind="ExternalInput").ap()
        return din[name]

    dt_in("x", [DIM, HW])
    dt_in("qw_t", [DIM, DIM])
    dt_in("kwk_t", [DIM, DIM])
    dt_in("kwv_t", [DIM, DIM])
    dt_in("pw_t", [NGD, 3])
    dt_in("projw_t", [DIM, DIM])
    dt_in("projb_rs", [NGD, 3])
    dt_in("dww", [NGD, 25])
    dt_in("bn_s", [NGD, 1])
    dt_in("bn_t", [NGD, 1])
    dt_in("ident", [128, 128])
    dt_in("ytab", [LC, 21])
    dt_in("xtab", [LC, 21])

    out_d = nc.dram_tensor("out", [DIM, HW], F32, kind="ExternalOutput").ap()

    with tile.TileContext(nc) as tc:
        _body(nc, tc, din, out_d, gelu_exact, stop_after)

    nc.compile()
    return nc


def _body(nc, tc, din, out_d, gelu_exact, stop_after=""):
    import contextlib
    ctx = contextlib.ExitStack()
    with ctx:
        # persistent pools (whole kernel)
        wpool = ctx.enter_context(tc.tile_pool(name="wpool", bufs=1))
        spool = ctx.enter_context(tc.tile_pool(name="spool", bufs=1))
        qpool = ctx.enter_context(tc.tile_pool(name="qpool", bufs=1))
        psum = ctx.enter_context(tc.tile_pool(name="psum", bufs=1, space="PSUM"))
        dram = ctx.enter_context(tc.tile_pool(name="dram", bufs=1, space="DRAM"))

        # ---------------- phase A: weight loads ----------------
        # fp32r matmul operands must be produced by compute ops (DMA does not
        # round to fp32r), so every DMA-loaded matmul operand goes through a
        # conversion copy into an F32R tile.
        def load_small(key, shape, dtype=F32):
            t = spool.tile(shape, dtype, name=key + "_sb")
            nc.sync.dma_start(t[:], din[key][:])
            return t

        pjb_sb = load_small("projb_rs", [NGD, 3])
        dww_sb = load_small("dww", [NGD, 25])
        bns_sb = load_small("bn_s", [NGD, 1])
        bnt_sb = load_small("bn_t", [NGD, 1])
        idn_sb = load_small("ident", [128, 128])
        ytab_sb = load_small("ytab", [LC, 21])
        xtab_sb = load_small("xtab", [LC, 21])

        # x pool: released after the gather phase
        xctx = contextlib.ExitStack()
        xpool = xctx.enter_context(tc.tile_pool(name="xpool", bufs=1))
        x_sb = [xpool.tile([128, HW], F32R, name=f"x_sb{g}") for g in range(G)]
        qw_sb, kwk_sb, kwv_sb, pjw_sb = [], [], [], []
        with tc.tile_pool(name="ldpool", bufs=1) as ldpool:
            for g in range(G):
                xt = ldpool.tile([128, HW], F32, tag="xtmp", bufs=2, name="xt")
                nc.sync.dma_start(xt[:], din["x"][128 * g:128 * (g + 1), :])
                nc.scalar.activation(x_sb[g][:], xt[:], AF.Copy)
            for name, key, dst in (("qw", "qw_t", qw_sb), ("kwk", "kwk_t", kwk_sb),
                                   ("kwv", "kwv_t", kwv_sb), ("pjw", "projw_t", pjw_sb)):
                for kc in range(3):
                    wt = ldpool.tile([128, DIM], F32, tag="wtmp", bufs=4, name="wt")
                    nc.sync.dma_start(wt[:], din[key][128 * kc:128 * (kc + 1), :])
                    t = wpool.tile([128, DIM], F32R, name=f"{name}_r{kc}")
                    nc.vector.tensor_copy(t[:], wt[:])
                    dst.append(t)
            pw_sb = spool.tile([NGD, 3], F32, name="pw_sb")
            nc.sync.dma_start(pw_sb[:], din["pw_t"][:])

        ones64 = spool.tile([1, 64], F32R, name="ones64")
        nc.vector.memset(ones64[:].bitcast(F32), 1.0)
        ones128 = spool.tile([1, 128], F32R, name="ones128")
        nc.vector.memset(ones128[:].bitcast(F32), 1.0)

        # ---------------- phase B: q = q_w @ x ----------------
        q_sb = [qpool.tile([128, HW], F32R, name=f"q_sb{m}") for m in range(3)]
        for m in range(3):
            for n in range(NQC):
                pq = psum.tile([128, QC], F32, tag="big", bufs=2, name="pq")
                for kc in range(3):
                    nc.tensor.matmul(
                        pq[:],
                        _r(qw_sb[kc][:, 128 * m:128 * (m + 1)]),
                        _r(x_sb[kc][:, QC * n:QC * (n + 1)]),
                        start=(kc == 0), stop=(kc == 2),
                    )
                nc.vector.tensor_copy(q_sb[m][:, QC * n:QC * (n + 1)], pq[:])

        def _dump(tiles):
            for mm, tt in enumerate(tiles[:3]):
                nc.sync.dma_start(out_d[128 * mm:128 * (mm + 1), 0:tt.shape[1]],
                                  tt[:].bitcast(F32) if tt.dtype != F32 else tt[:])

        if stop_after == "B":
            _dump(q_sb)
            xctx.close()
            return

        # ---------------- phases C..G: per-group pipelined ----------------
        idx_dr = dram.tile([G * 4 * NLC * LC], I16)    # flat (g, r, c, p)
        wgt_dr = dram.tile([G * 4 * NLC * LC], F32)
        idx_v = idx_dr.rearrange("(g p r c) -> g p r c", g=G, p=0, r=4, c=NLC) \
            if False else idx_dr.rearrange("(g r c p) -> g p r c", g=G, r=4, c=NLC)
        wgt_v = wgt_dr.rearrange("(g r c p) -> g p r c", g=G, r=4, c=NLC)
        wrap_v = idx_dr.rearrange("(g s q) -> g q s", g=G, q=16)
        wrow_v = wgt_dr.rearrange("(g r n) -> g r n", g=G, r=4)

        xs_sb = [qpool.tile([128, L], F32R, name=f"xs_sb{g}") for g in range(G)]
        idxw = [spool.tile([128, 196], I16, name=f"idxw{g}") for g in range(G)]

        dgctx = __import__("contextlib").ExitStack()
        dgpool = dgctx.enter_context(tc.tile_pool(name="dgpool", bufs=1))
        diag = dgpool.tile([128, 25 * 128], F32R, name="diag")
        for t in range(25):
            nc.scalar.activation(
                diag[:, 128 * t:128 * (t + 1)], idn_sb[:], AF.Copy,
                scale=dww_sb[:, t:t + 1],
            )

        with tc.tile_pool(name="cpool", bufs=1) as cpool:
            def ctile(shape, dtype, tag, bufs=2):
                return cpool.tile(shape, dtype, tag=tag, bufs=bufs, name=tag)

            for g in range(G):
                # --- C1: padded input ---
                pad = ctile([128, PADD * PADD], F32R, "pad", bufs=2)
                nc.vector.memset(pad[:].bitcast(F32), 0.0)
                pad_v = pad[:].rearrange("p (h w) -> p h w", w=PADD)
                qv = q_sb[g][:].rearrange("p (h w) -> p h w", w=W)
                nc.vector.tensor_copy(pad_v[:, 2:58, 2:58], qv[:])

                # --- C3+C4: depthwise conv + BN + GELU ---
                gelu = ctile([128, L], F32, "gelu", bufs=2)
                for nn in range(2):
                    pdw = psum.tile([128, 392], F32, tag="big", bufs=2, name="pdw")
                    for t in range(25):
                        ty, tx = t // 5, t % 5
                        rhs = pad_v[:, ty + 28 * nn: ty + 28 * nn + 28: 2, tx: tx + 56: 2]
                        nc.tensor.matmul(
                            pdw[:], _r(diag[:, 128 * t:128 * (t + 1)]), _r(rhs),
                            start=(t == 0), stop=(t == 24),
                        )
                    gout = gelu[:, 392 * nn:392 * (nn + 1)]
                    if gelu_exact:
                        nc.scalar.activation(gout, pdw[:], AF.Gelu,
                                             bias=bnt_sb[:, 0:1], scale=bns_sb[:, 0:1])
                    else:
                        aa = ctile([128, 392], F32, "simg1")
                        nc.scalar.activation(aa[:], pdw[:], AF.Identity,
                                             bias=bnt_sb[:, 0:1], scale=bns_sb[:, 0:1])
                        ss = ctile([128, 392], F32, "simg2")
                        nc.scalar.activation(ss[:], aa[:], AF.Sigmoid, scale=1.702)
                        nc.vector.tensor_tensor(gout, aa[:], ss[:], op=OP.mult)

                # --- C5: om^T = gelu^T @ pw -> [112 pos, (chunk, ch)] ---
                pom = psum.tile([LC, 21], F32, tag="s", bufs=3, name="pom")
                for c in range(NLC):
                    nc.tensor.matmul(
                        pom[:, 3 * c:3 * (c + 1)],
                        gelu[:, LC * c:LC * (c + 1)],
                        pw_sb[:, 0:3],
                        start=True, stop=True,
                    )
                om_g = ctile([LC, 21], F32, "om_g")
                nc.vector.tensor_copy(om_g[:], pom[:])

                # --- D: position math on [112, 7] slices ---
                om_v = om_g[:].rearrange("p (k ch) -> p k ch", ch=3)
                om0, om1, om2 = om_v[:, :, 0], om_v[:, :, 1], om_v[:, :, 2]
                yt = ytab_sb[:, 0:NLC]
                xt = xtab_sb[:, 0:NLC]

                def dvt(tag):
                    return ctile([LC, NLC], F32, tag)

                ty_t = dvt("ty_t"); tx_t = dvt("tx_t"); mod_t = dvt("mod_t")
                nc.scalar.activation(ty_t[:], om0, AF.Tanh)
                nc.scalar.activation(tx_t[:], om1, AF.Tanh)
                sg_t = dvt("sg_t")
                nc.scalar.activation(sg_t[:], om2, AF.Sigmoid)
                nc.scalar.activation(mod_t[:], sg_t[:], AF.Sigmoid)

                gy2 = dvt("gy2"); gx2 = dvt("gx2")
                nc.vector.tensor_tensor(gy2[:], ty_t[:], yt, op=OP.add)
                nc.vector.tensor_scalar(gy2[:], gy2[:], float(A), None, OP.mult)
                nc.vector.tensor_tensor(gx2[:], tx_t[:], xt, op=OP.add)
                nc.vector.tensor_scalar(gx2[:], gx2[:], float(A), None, OP.mult)

                def floor_of(gt, tag):
                    ii = ctile([LC, NLC], I32, tag + "_i")
                    nc.vector.tensor_copy(ii[:], gt[:])
                    ff = dvt(tag + "_f")
                    nc.vector.tensor_copy(ff[:], ii[:])
                    fxm = dvt(tag + "_fix")
                    nc.vector.tensor_tensor(fxm[:], ff[:], gt[:], op=OP.is_gt)
                    nc.vector.tensor_tensor(ff[:], ff[:], fxm[:], op=OP.subtract)
                    return ff

                y0s = floor_of(gy2, "y0s")
                x0s = floor_of(gx2, "x0s")

                fy = dvt("fy"); fx_ = dvt("fx_")
                nc.vector.tensor_tensor(fy[:], gy2[:], y0s[:], op=OP.subtract)
                nc.vector.tensor_tensor(fx_[:], gx2[:], x0s[:], op=OP.subtract)

                my0 = dvt("my0"); my1 = dvt("my1"); mx0 = dvt("mx0"); mx1 = dvt("mx1")
                nc.vector.tensor_scalar(my0[:], gy2[:], 2.0, None, OP.is_ge)
                nc.vector.tensor_scalar(my1[:], gy2[:], 57.0, None, OP.is_lt)
                nc.vector.tensor_scalar(mx0[:], gx2[:], 2.0, None, OP.is_ge)
                nc.vector.tensor_scalar(mx1[:], gx2[:], 57.0, None, OP.is_lt)

                wy0 = dvt("wy0"); wy1 = dvt("wy1"); wx0 = dvt("wx0"); wx1 = dvt("wx1")
                omf = dvt("omf")
                nc.vector.tensor_scalar(omf[:], fy[:], -1.0, 1.0, OP.mult, OP.add)
                nc.vector.tensor_tensor(wy0[:], omf[:], my0[:], op=OP.mult)
                nc.vector.tensor_tensor(wy0[:], wy0[:], mod_t[:], op=OP.mult)
                nc.vector.tensor_tensor(wy1[:], fy[:], my1[:], op=OP.mult)
                nc.vector.tensor_tensor(wy1[:], wy1[:], mod_t[:], op=OP.mult)
                nc.vector.tensor_scalar(omf[:], fx_[:], -1.0, 1.0, OP.mult, OP.add)
                nc.vector.tensor_tensor(wx0[:], omf[:], mx0[:], op=OP.mult)
                nc.vector.tensor_tensor(wx1[:], fx_[:], mx1[:], op=OP.mult)

                Wt_g = ctile([LC, 4 * NLC], F32, "Wt_g")
                Wv = Wt_g[:].rearrange("p (r c) -> p r c", r=4)
                nc.vector.tensor_tensor(Wv[:, 0, :], wy0[:], wx0[:], op=OP.mult)
                nc.vector.tensor_tensor(Wv[:, 1, :], wy0[:], wx1[:], op=OP.mult)
                nc.vector.tensor_tensor(Wv[:, 2, :], wy1[:], wx0[:], op=OP.mult)
                nc.vector.tensor_tensor(Wv[:, 3, :], wy1[:], wx1[:], op=OP.mult)

                yc0 = dvt("yc0"); yc1 = dvt("yc1"); xc0 = dvt("xc0"); xc1 = dvt("xc1")
                nc.vector.tensor_scalar(yc0[:], y0s[:], -2.0, 0.0, OP.add, OP.max)
                nc.vector.tensor_scalar(yc0[:], yc0[:], 55.0, 56.0, OP.min, OP.mult)
                nc.vector.tensor_scalar(yc1[:], y0s[:], -1.0, 0.0, OP.add, OP.max)
                nc.vector.tensor_scalar(yc1[:], yc1[:], 55.0, 56.0, OP.min, OP.mult)
                nc.vector.tensor_scalar(xc0[:], x0s[:], -2.0, 0.0, OP.add, OP.max)
                nc.vector.tensor_scalar(xc0[:], xc0[:], 55.0, None, OP.min)
                nc.vector.tensor_scalar(xc1[:], x0s[:], -1.0, 0.0, OP.add, OP.max)
                nc.vector.tensor_scalar(xc1[:], xc1[:], 55.0, None, OP.min)

                If_g = ctile([LC, 4 * NLC], F32, "If_g")
                Ifv = If_g[:].rearrange("p (r c) -> p r c", r=4)
                nc.vector.tensor_tensor(Ifv[:, 0, :], yc0[:], xc0[:], op=OP.add)
                nc.vector.tensor_tensor(Ifv[:, 1, :], yc0[:], xc1[:], op=OP.add)
                nc.vector.tensor_tensor(Ifv[:, 2, :], yc1[:], xc0[:], op=OP.add)
                nc.vector.tensor_tensor(Ifv[:, 3, :], yc1[:], xc1[:], op=OP.add)
                Ii_g = ctile([LC, 4 * NLC], I16, "Ii_g")
                nc.vector.tensor_copy(Ii_g[:], If_g[:])

                # --- E: DRAM wrap roundtrip ---
                nc.sync.dma_start(idx_v[g], Ii_g[:])
                nc.sync.dma_start(wgt_v[g], Wt_g[:])
                for gi in range(8):
                    nc.sync.dma_start(idxw[g][16 * gi:16 * (gi + 1), :], wrap_v[g])

                wbc = []
                for r in range(4):
                    wrow_f = ctile([1, L], F32, "wrow_f", bufs=1)
                    nc.sync.dma_start(wrow_f[:], wrow_v[g, r][None, :])
                    wrow = ctile([1, L], F32R, "wrow", bufs=1)
                    nc.vector.tensor_copy(wrow[:], wrow_f[:])
                    t = ctile([128, L], F32, "wbc", bufs=4)
                    for n2 in range(2):
                        pwb = psum.tile([128, 392], F32, tag="big", bufs=2, name="pwb")
                        nc.tensor.matmul(
                            pwb[:], ones128[:],
                            wrow[:, 392 * n2:392 * (n2 + 1)],
                            start=True, stop=True,
                        )
                        nc.scalar.activation(t[:, 392 * n2:392 * (n2 + 1)], pwb[:], AF.Copy)
                    wbc.append(t)

                # --- F+G: gather + bilinear ---
                gat = ctile([128, 4 * L], F32, "gat", bufs=2)
                nc.gpsimd.ap_gather(
                    gat[:], x_sb[g][:].bitcast(F32), idxw[g][:],
                    channels=128, num_elems=HW, d=1, num_idxs=4 * L,
                )
                tmp = ctile([128, L], F32, "biltmp", bufs=1)
                nc.vector.tensor_tensor(xs_sb[g][:], gat[:, 0:L], wbc[0][:], op=OP.mult)
                for r in range(1, 4):
                    nc.vector.tensor_tensor(tmp[:], gat[:, L * r:L * (r + 1)],
                                            wbc[r][:], op=OP.mult)
                    nc.vector.tensor_tensor(xs_sb[g][:], xs_sb[g][:], tmp[:], op=OP.add)

        dgctx.close()
        xctx.close()   # release x tiles
        if stop_after == "G":
            _dump(xs_sb)
            return

        # ---------------- phase H: k and v^T ----------------
        hpool = ctx.enter_context(tc.tile_pool(name="hpool", bufs=1))
        k_sb = [hpool.tile([128, L], F32R, name=f"k_sb{m}") for m in range(3)]
        for m in range(3):
            for n2 in range(2):
                pk = psum.tile([128, 392], F32, tag="big", bufs=2, name="pk")
                for kc in range(3):
                    nc.tensor.matmul(
                        pk[:],
                        _r(kwk_sb[kc][:, 128 * m:128 * (m + 1)]),
                        _r(xs_sb[kc][:, 392 * n2:392 * (n2 + 1)]),
                        start=(kc == 0), stop=(kc == 2),
                    )
                nc.scalar.activation(k_sb[m][:, 392 * n2:392 * (n2 + 1)], pk[:], AF.Copy)

        vTe = [hpool.tile([LC, 6 * 65], F32R, name=f"vTe{lc}") for lc in range(NLC)]
        for lc in range(NLC):
            nc.vector.memset(vTe[lc][:].bitcast(F32), 1.0)
            pv = psum.tile([LC, DIM], F32, tag="big", bufs=2, name="pv")
            for kc in range(3):
                nc.tensor.matmul(
                    pv[:],
                    _r(xs_sb[kc][:, LC * lc:LC * (lc + 1)]),
                    _r(kwv_sb[kc][:, 0:DIM]),
                    start=(kc == 0), stop=(kc == 2),
                )
            dst = vTe[lc][:].rearrange("p (h d) -> p h d", h=6)[:, :, 0:64]
            nc.scalar.activation(dst, pv[:].rearrange("p (h d) -> p h d", h=6), AF.Copy)

        if stop_after == "H":
            _dump(k_sb)
            return

        # ---------------- phases I+J ----------------
        with tc.tile_pool(name="opool", bufs=1) as opool, \
             tc.tile_pool(name="apool", bufs=1) as apool:
            O_all = [opool.tile([128, HW], F32R, name=f"O_all{m}") for m in range(3)]

            def st_phase(h, qi):
                m2, hh = h // 2, h % 2
                Es = []
                for lc in range(NLC):
                    ps_s = psum.tile([LC, QC], F32, tag="s", bufs=3, name="ps_s")
                    nc.tensor.matmul(
                        ps_s[:],
                        _r(k_sb[m2][64 * hh:64 * hh + 64, LC * lc:LC * (lc + 1)]),
                        _r(q_sb[m2][64 * hh:64 * hh + 64, QC * qi:QC * (qi + 1)]),
                        start=True, stop=True,
                    )
                    E = apool.tile([LC, QC], F32R, tag="E", bufs=16, name="E")
                    nc.scalar.activation(E[:], ps_s[:], AF.Exp)
                    Es.append(E)
                return Es

            def ot_phase(h, qi, Es):
                m2, hh = h // 2, h % 2
                ps_o = psum.tile([65, QC], F32, tag="o", bufs=3, name="ps_o")
                for lc in range(NLC):
                    nc.tensor.matmul(
                        ps_o[:],
                        _r(vTe[lc][:, 65 * h:65 * (h + 1)]),
                        _r(Es[lc][:]),
                        start=(lc == 0), stop=(lc == NLC - 1),
                    )
                rec = apool.tile([1, QC], F32R, tag="rec", bufs=4, name="rec")
                with nc.allow_low_precision(reason="f32r is fp32-width"):
                    nc.vector.reciprocal(rec[:], ps_o[64:65, :])
                ps_rb = psum.tile([64, QC], F32, tag="o", bufs=3, name="ps_rb")
                nc.tensor.matmul(ps_rb[:], ones64[:], rec[:],
                                 start=True, stop=True)
                oslice = O_all[m2][64 * hh:64 * hh + 64, QC * qi:QC * (qi + 1)]
                nc.vector.tensor_copy(oslice, ps_o[0:64, :])
                nc.vector.tensor_tensor(oslice, oslice, ps_rb[:], op=OP.mult)

            attn_iters = [(h, qi) for h in range(NUM_HEAD) for qi in range(NQC)]
            if stop_after.startswith("I1"):
                attn_iters = attn_iters[:1]
            pending = None
            for it in attn_iters:
                Es = st_phase(*it)
                if pending is not None:
                    ot_phase(pending[0][0], pending[0][1], pending[1])
                pending = (it, Es)
            if pending is not None:
                ot_phase(pending[0][0], pending[0][1], pending[1])

            if stop_after.startswith("I1") or stop_after == "I":
                _dump(O_all[:1] if stop_after.startswith("I1") else O_all)
                return

            # proj
            for m in range(3):
                for n in range(NQC):
                    pp = psum.tile([128, QC], F32, tag="big", bufs=2, name="pp")
                    for kc in range(3):
                        nc.tensor.matmul(
                            pp[:],
                            _r(pjw_sb[kc][:, 128 * m:128 * (m + 1)]),
                            _r(O_all[kc][:, QC * n:QC * (n + 1)]),
                            start=(kc == 0), stop=(kc == 2),
                        )
                    y = apool.tile([128, QC], F32, tag="y", bufs=3, name="y")
                    nc.vector.tensor_scalar(y[:], pp[:], pjb_sb[:, m:m + 1], None,
                                            OP.add)
                    nc.sync.dma_start(
                        out_d[128 * m:128 * (m + 1), QC * n:QC * (n + 1)], y[:])


def host_prep(inputs):
    """Shared (per-core-identical) weight prep. Returns dict of np arrays."""
    f = np.float32
    q_w = np.asarray(inputs["q_w"], f)
    kv_w = np.asarray(inputs["kv_w"], f)
    proj_w = np.asarray(inputs["proj_w"], f)
    proj_b = np.asarray(inputs["proj_b"], f)
    dw_w = np.asarray(inputs["dw_w"], f)
    dw_b = np.asarray(inputs["dw_b"], f)
    bn_w = np.asarray(inputs["bn_w"], f)
    bn_b = np.asarray(inputs["bn_b"], f)
    bn_mean = np.asarray(inputs["bn_mean"], f)
    bn_var = np.asarray(inputs["bn_var"], f)
    pw_w = np.asarray(inputs["pw_w"], f)

    bn_s = (bn_w / np.sqrt(bn_var + BN_EPS)).astype(f)
    bn_t = ((dw_b - bn_mean) * bn_s + bn_b).astype(f)

    p = np.arange(LC)
    c = np.arange(NLC)
    ytab_col = (4 * c[None, :] + p[:, None] // 28 + 0.5 + 2.0 / A).astype(f)  # [112, 7]
    ytab = np.tile(ytab_col, (1, G))                                          # [112, 21]
    xtab_col = (p % 28 + 0.5 + 2.0 / A).astype(f)[:, None]
    xtab = np.tile(xtab_col, (1, G * NLC))

    return {
        "qw_t": np.ascontiguousarray(q_w.T),
        "kwk_t": np.ascontiguousarray((kv_w[:DIM] * SCALE).T),
        "kwv_t": np.ascontiguousarray(kv_w[DIM:].T),
        "pw_t": np.ascontiguousarray(pw_w.T),
        "projw_t": np.ascontiguousarray(proj_w.T),
        "projb_rs": np.ascontiguousarray(proj_b.reshape(3, NGD).T),
        "dww": np.ascontiguousarray(dw_w.reshape(NGD, 25)),
        "bn_s": bn_s.reshape(NGD, 1),
        "bn_t": bn_t.reshape(NGD, 1),
        "ident": np.eye(128, dtype=f),
        "ytab": ytab,
        "xtab": xtab,
    }


_NC_CACHE = {}


def _get_nc(gelu_exact=True):
    key = bool(gelu_exact)
    if key not in _NC_CACHE:
        _NC_CACHE[key] = build_nc(gelu_exact=key)
    return _NC_CACHE[key]


def make_in_maps(inputs):
    shared = host_prep(inputs)
    x = np.asarray(inputs["x"], np.float32)
    in_maps = []
    for i in range(B):
        m = dict(shared)
        m["x"] = np.ascontiguousarray(x[i].reshape(DIM, HW))
        in_maps.append(m)
    return in_maps


def run_spmd(inputs, trace=False):
    """Run on the 8 NeuronCores; returns (out (8,384,56,56), BassKernelResults)."""
    nc = _get_nc(True)
    in_maps = make_in_maps(inputs)
    res = bass_utils.run_bass_kernel_spmd(
        nc, in_maps, core_ids=list(range(B)), trace=trace,
    )
    out = np.stack([r["out"].reshape(DIM, H, W) for r in res.results], axis=0)
    return out, res


def kernel(**inputs) -> np.ndarray:
    out, _ = run_spmd(inputs, trace=False)
    return out



# revision 9
# speedup vs baseline: 1.2848x; 1.2848x over previous
"""Deformable multi-head sparse attention (DMSA) Bass kernel for Trainium2.

Contract: kernel(**inputs) takes the FULL unsharded inputs (as produced by
setup_inputs()) and returns the FULL output (B, 384, 56, 56) float32.
Internally shards batch B=8 across 8 NeuronCores (pure data parallel,
no collectives), one batch element per core.

Self-contained: hardcodes all shapes; does not read any sibling files.
"""
import sys

for _p in ("/opt/trn_rl_repo", "/opt/pypackages"):
    if _p not in sys.path:
        sys.path.insert(0, _p)

import numpy as np
import ml_dtypes

import concourse.bass as bass
import concourse.mybir as mybir
import concourse.tile as tile
from concourse import bacc
from concourse import bass_utils

F32 = mybir.dt.float32
F32R = mybir.dt.float32r
BF16 = mybir.dt.bfloat16
I16 = mybir.dt.int16
I32 = mybir.dt.int32
AF = mybir.ActivationFunctionType
OP = mybir.AluOpType

# problem constants
B = 8
DIM = 384
DIM_HEAD = 64
NUM_HEAD = 6
G = 3            # deformable groups
NGD = 128        # channels per group
H = 56
W = 56
HW = H * W       # 3136
HO = 28
WO = 28
L = HO * WO      # 784
SCALE = DIM_HEAD ** -0.5
BN_EPS = 1e-6
A = (W - 1) / WO   # 55/28, same for y since H==W and HO==WO
PADD = 60          # padded dwconv input edge (56 + 2*2)

QC = 448           # q-position chunk (free dim of attention matmuls)
NQC = HW // QC     # 7
LC = 112           # kv-position chunk (partition dim of S^T)
NLC = L // LC      # 7
QB = 512           # psum bank stride (f32 elems)

# Schraudolph exp-via-bf16-bits constants (float->int16 truncation)
SCH_A = 184.66496
SCH_B = 16250.90
# kv tiles computed on Pool with Schraudolph exp (rest on ACT, exact)
POOL_LCS = (2, 3)


def build_nc(gelu_exact: bool = True):
    """Build the per-core Bass program (SPMD: same NEFF on all 8 cores)."""
    nc = bacc.Bacc("TRN2", target_bir_lowering=False, debug=False, num_devices=B)

    din = {}
    def dt_in(name, shape, dtype=F32):
        din[name] = nc.dram_tensor(name, shape, dtype, kind="ExternalInput").ap()
        return din[name]

    dt_in("x", [DIM, HW])
    dt_in("qw_t", [DIM, DIM], BF16)
    dt_in("kwk_t", [DIM, DIM], BF16)
    dt_in("kwv_t", [DIM, DIM], BF16)
    dt_in("projw_t", [DIM, DIM], BF16)
    dt_in("pjb_row", [1, DIM], BF16)
    dt_in("pw_t", [NGD, 3])
    dt_in("diag", [NGD, 25 * 128], BF16)
    dt_in("ind6", [NUM_HEAD, DIM], F32R)
    dt_in("bn_s", [NGD, 1])
    dt_in("bn_t", [NGD, 1])
    dt_in("ytab", [LC, 21])
    dt_in("xtab", [LC, 21])

    out_d = nc.dram_tensor("out", [DIM, HW], F32, kind="ExternalOutput").ap()

    with tile.TileContext(nc) as tc:
        _body(nc, tc, din, out_d)

    nc.compile()
    return nc


def _body(nc, tc, din, out_d):
    import contextlib
    ctx = contextlib.ExitStack()
    with ctx:
        # persistent pools (whole kernel)
        wpool = ctx.enter_context(tc.tile_pool(name="wpool", bufs=1))
        spool = ctx.enter_context(tc.tile_pool(name="spool", bufs=1))
        qpool = ctx.enter_context(tc.tile_pool(name="qpool", bufs=1))
        dram = ctx.enter_context(tc.tile_pool(name="dram", bufs=1, space="DRAM"))

        # ---------------- phase A: weight loads (all bf16, direct DMA) -----
        def load_w(key, shape, dtype=BF16):
            t = wpool.tile(shape, dtype, name=key + "_sb")
            nc.sync.dma_start(t[:], din[key][:])
            return t

        def load_w3(key, dtype=BF16):
            ts = []
            for kc in range(3):
                t = wpool.tile([128, DIM], dtype, name=f"{key}_sb{kc}")
                nc.sync.dma_start(t[:], din[key][128 * kc:128 * (kc + 1), :])
                ts.append(t)
            return ts

        qw_v = load_w3("qw_t")
        kwk_v = load_w3("kwk_t")
        kwv_v = load_w3("kwv_t")
        pjw_v = load_w3("projw_t")
        pjb_sb = load_w("pjb_row", [1, DIM])
        diag_sb = load_w("diag", [NGD, 25 * 128])
        ind_sb = load_w("ind6", [NUM_HEAD, DIM], F32R)
        pw_sb = load_w("pw_t", [NGD, 3], F32)
        bns_sb = load_w("bn_s", [NGD, 1], F32)
        bnt_sb = load_w("bn_t", [NGD, 1], F32)
        ytab_sb = load_w("ytab", [LC, 21], F32)
        xtab_sb = load_w("xtab", [LC, 21], F32)

        ones128 = spool.tile([1, 128], BF16, name="ones128")
        nc.vector.memset(ones128[:], 1.0)
        ones448 = spool.tile([1, QC], BF16, name="ones448")
        nc.vector.memset(ones448[:], 1.0)

        # x pool: released after the gather phase
        xctx = contextlib.ExitStack()
        xpool = xctx.enter_context(tc.tile_pool(name="xpool", bufs=1))
        x_sb = [xpool.tile([128, HW], BF16, name=f"x_sb{g}") for g in range(G)]
        xt_sb = [xpool.tile([128, HW], F32, name=f"xt_sb{g}") for g in range(G)]
        for g in range(G):
            nc.sync.dma_start(xt_sb[g][:], din["x"][128 * g:128 * (g + 1), :])
            nc.vector.tensor_copy(x_sb[g][:], xt_sb[g][:])

        # pre-attention psum pool
        prectx = contextlib.ExitStack()
        psum = prectx.enter_context(tc.tile_pool(name="psum", bufs=1, space="PSUM"))

        # ---------------- phase B: q = q_w @ x ----------------
        q_sb = [qpool.tile([128, HW], BF16, name=f"q_sb{m}") for m in range(3)]
        for m in range(3):
            for n in range(NQC):
                pq = psum.tile([128, QC], F32, tag="big", bufs=2, name="pq")
                for kc in range(3):
                    nc.tensor.matmul(
                        pq[:],
                        qw_v[kc][:, 128 * m:128 * (m + 1)],
                        x_sb[kc][:, QC * n:QC * (n + 1)],
                        start=(kc == 0), stop=(kc == 2),
                    )
                nc.scalar.activation(q_sb[m][:, QC * n:QC * (n + 1)], pq[:], AF.Copy)

        # ---------------- phases C..G ----------------
        idx_dr = dram.tile([G * 4 * NLC * LC], I16)    # flat (g, r, c, p)
        wgt_dr = dram.tile([G * 4 * NLC * LC], BF16)
        idx_v = idx_dr.rearrange("(g r c p) -> g p r c", g=G, r=4, c=NLC)
        wgt_v = wgt_dr.rearrange("(g r c p) -> g p r c", g=G, r=4, c=NLC)
        wrap_v = idx_dr.rearrange("(g s q) -> g q s", g=G, q=16)
        wrow_v = wgt_dr.rearrange("(g r n) -> g r n", g=G, r=4)

        xs_sb = [qpool.tile([128, L], BF16, name=f"xs_sb{g}") for g in range(G)]
        idxw = [spool.tile([128, 196], I16, name=f"idxw{g}") for g in range(G)]

        with tc.tile_pool(name="cpool", bufs=1) as cpool:
            def ctile(shape, dtype, tag, bufs=2):
                return cpool.tile(shape, dtype, tag=tag, bufs=bufs, name=tag)

            # --- dwconv + BN + GELU + pw per group; om batched across g ---
            pom = psum.tile([LC, 63], F32, tag="pom", bufs=1, name="pom")
            gelus = []
            for g in range(G):
                pad = ctile([128, PADD * PADD], BF16, "pad", bufs=2)
                pad_v = pad[:].rearrange("p (h w) -> p h w", w=PADD)
                nc.vector.memset(pad_v[:, 0:2, :], 0.0)
                nc.vector.memset(pad_v[:, 58:60, :], 0.0)
                nc.vector.memset(pad_v[:, 2:58, 0:2], 0.0)
                nc.vector.memset(pad_v[:, 2:58, 58:60], 0.0)
                qv = q_sb[g][:].rearrange("p (h w) -> p h w", w=W)
                nc.vector.tensor_copy(pad_v[:, 2:58, 2:58], qv[:])

                gelu = ctile([128, L], F32, "gelu", bufs=2)
                gelus.append(gelu)
                for nn in range(2):
                    pdw = psum.tile([128, 392], F32, tag="pdw", bufs=2, name="pdw")
                    for t in range(25):
                        ty, tx = t // 5, t % 5
                        rhs = pad_v[:, ty + 28 * nn: ty + 28 * nn + 28: 2, tx: tx + 56: 2]
                        nc.tensor.matmul(
                            pdw[:], diag_sb[:, 128 * t:128 * (t + 1)], rhs,
                            start=(t == 0), stop=(t == 24),
                        )
                    nc.scalar.activation(gelu[:, 392 * nn:392 * (nn + 1)], pdw[:],
                                         AF.Gelu, bias=bnt_sb[:, 0:1],
                                         scale=bns_sb[:, 0:1])
                # om^T chunks: [112, 3] per (g, c) into pom[:, g*21+3c : +3]
                for c in range(NLC):
                    nc.tensor.matmul(
                        pom[:, 21 * g + 3 * c:21 * g + 3 * c + 3],
                        gelu[:, LC * c:LC * (c + 1)],
                        pw_sb[:, 0:3],
                        start=True, stop=True,
                    )

            # --- position math, batched across all 3 groups: [112, 21] ---
            om_all = ctile([LC, 63], F32, "om_all", bufs=1)
            nc.vector.tensor_copy(om_all[:], pom[:])
            om_v = om_all[:].rearrange("p (k ch) -> p k ch", ch=3)  # k = (g, c)
            om0, om1, om2 = om_v[:, :, 0], om_v[:, :, 1], om_v[:, :, 2]
            yt = ytab_sb[:, 0:21]
            xt = xtab_sb[:, 0:21]

            def dvt(tag):
                return ctile([LC, 21], F32, tag, bufs=1)

            ty_t = dvt("ty_t"); tx_t = dvt("tx_t"); mod_t = dvt("mod_t")
            nc.scalar.activation(ty_t[:], om0, AF.Tanh)
            nc.scalar.activation(tx_t[:], om1, AF.Tanh)
            sg_t = dvt("sg_t")
            nc.scalar.activation(sg_t[:], om2, AF.Sigmoid)
            nc.scalar.activation(mod_t[:], sg_t[:], AF.Sigmoid)

            gy2 = dvt("gy2"); gx2 = dvt("gx2")
            nc.vector.tensor_tensor(gy2[:], ty_t[:], yt, op=OP.add)
            nc.vector.tensor_scalar(gy2[:], gy2[:], float(A), None, OP.mult)
            nc.vector.tensor_tensor(gx2[:], tx_t[:], xt, op=OP.add)
            nc.vector.tensor_scalar(gx2[:], gx2[:], float(A), None, OP.mult)

            def floor_of(gt, tag):
                ii = ctile([LC, 21], I32, tag + "_i", bufs=1)
                nc.vector.tensor_copy(ii[:], gt[:])
                ff = dvt(tag + "_f")
                nc.vector.tensor_copy(ff[:], ii[:])
                fxm = dvt(tag + "_fix")
                nc.vector.tensor_tensor(fxm[:], ff[:], gt[:], op=OP.is_gt)
                nc.vector.tensor_tensor(ff[:], ff[:], fxm[:], op=OP.subtract)
                return ff

            y0s = floor_of(gy2, "y0s")
            x0s = floor_of(gx2, "x0s")

            fy = dvt("fy"); fx_ = dvt("fx_")
            nc.vector.tensor_tensor(fy[:], gy2[:], y0s[:], op=OP.subtract)
            nc.vector.tensor_tensor(fx_[:], gx2[:], x0s[:], op=OP.subtract)

            my0 = dvt("my0"); my1 = dvt("my1"); mx0 = dvt("mx0"); mx1 = dvt("mx1")
            nc.vector.tensor_scalar(my0[:], gy2[:], 2.0, None, OP.is_ge)
            nc.vector.tensor_scalar(my1[:], gy2[:], 57.0, None, OP.is_lt)
            nc.vector.tensor_scalar(mx0[:], gx2[:], 2.0, None, OP.is_ge)
            nc.vector.tensor_scalar(mx1[:], gx2[:], 57.0, None, OP.is_lt)

            wy0 = dvt("wy0"); wy1 = dvt("wy1"); wx0 = dvt("wx0"); wx1 = dvt("wx1")
            omf = dvt("omf")
            nc.vector.tensor_scalar(omf[:], fy[:], -1.0, 1.0, OP.mult, OP.add)
            nc.vector.tensor_tensor(wy0[:], omf[:], my0[:], op=OP.mult)
            nc.vector.tensor_tensor(wy0[:], wy0[:], mod_t[:], op=OP.mult)
            nc.vector.tensor_tensor(wy1[:], fy[:], my1[:], op=OP.mult)
            nc.vector.tensor_tensor(wy1[:], wy1[:], mod_t[:], op=OP.mult)
            nc.vector.tensor_scalar(omf[:], fx_[:], -1.0, 1.0, OP.mult, OP.add)
            nc.vector.tensor_tensor(wx0[:], omf[:], mx0[:], op=OP.mult)
            nc.vector.tensor_tensor(wx1[:], fx_[:], mx1[:], op=OP.mult)

            # Wt layout [112, (g, r, c)]; slice per g is [112, (r, c)] = [112, 28]
            Wt_all = ctile([LC, 84], BF16, "Wt_all", bufs=1)
            Wv = Wt_all[:].rearrange("p (g r c) -> p g r c", g=G, r=4)
            If_all = ctile([LC, 84], F32, "If_all", bufs=1)
            Ifv = If_all[:].rearrange("p (g r c) -> p g r c", g=G, r=4)

            yc0 = dvt("yc0"); yc1 = dvt("yc1"); xc0 = dvt("xc0"); xc1 = dvt("xc1")
            nc.vector.tensor_scalar(yc0[:], y0s[:], -2.0, 0.0, OP.add, OP.max)
            nc.vector.tensor_scalar(yc0[:], yc0[:], 55.0, 56.0, OP.min, OP.mult)
            nc.vector.tensor_scalar(yc1[:], y0s[:], -1.0, 0.0, OP.add, OP.max)
            nc.vector.tensor_scalar(yc1[:], yc1[:], 55.0, 56.0, OP.min, OP.mult)
            nc.vector.tensor_scalar(xc0[:], x0s[:], -2.0, 0.0, OP.add, OP.max)
            nc.vector.tensor_scalar(xc0[:], xc0[:], 55.0, None, OP.min)
            nc.vector.tensor_scalar(xc1[:], x0s[:], -1.0, 0.0, OP.add, OP.max)
            nc.vector.tensor_scalar(xc1[:], xc1[:], 55.0, None, OP.min)

            # per-(g) views of the [112, 21] math tiles: columns g*7..g*7+7
            for g in range(G):
                s = slice(7 * g, 7 * (g + 1))
                nc.vector.tensor_tensor(Wv[:, g, 0, :], wy0[:, s], wx0[:, s], op=OP.mult)
                nc.vector.tensor_tensor(Wv[:, g, 1, :], wy0[:, s], wx1[:, s], op=OP.mult)
                nc.vector.tensor_tensor(Wv[:, g, 2, :], wy1[:, s], wx0[:, s], op=OP.mult)
                nc.vector.tensor_tensor(Wv[:, g, 3, :], wy1[:, s], wx1[:, s], op=OP.mult)
                nc.vector.tensor_tensor(Ifv[:, g, 0, :], yc0[:, s], xc0[:, s], op=OP.add)
                nc.vector.tensor_tensor(Ifv[:, g, 1, :], yc0[:, s], xc1[:, s], op=OP.add)
                nc.vector.tensor_tensor(Ifv[:, g, 2, :], yc1[:, s], xc0[:, s], op=OP.add)
                nc.vector.tensor_tensor(Ifv[:, g, 3, :], yc1[:, s], xc1[:, s], op=OP.add)
            Ii_all = ctile([LC, 84], I16, "Ii_all", bufs=1)
            nc.vector.tensor_copy(Ii_all[:], If_all[:])

            # --- DRAM wrap roundtrip + gather + bilinear, per group ---
            for g in range(G):
                nc.sync.dma_start(idx_v[g], Ii_all[:, 28 * g:28 * (g + 1)])
                nc.sync.dma_start(wgt_v[g], Wt_all[:, 28 * g:28 * (g + 1)])
                for gi in range(8):
                    nc.sync.dma_start(idxw[g][16 * gi:16 * (gi + 1), :], wrap_v[g])

                wbc = []
                for r in range(4):
                    wrow = ctile([1, L], BF16, "wrow", bufs=2)
                    nc.sync.dma_start(wrow[:], wrow_v[g, r][None, :])
                    t = ctile([128, L], F32, "wbc", bufs=4)
                    for n2 in range(2):
                        pwb = psum.tile([128, 392], F32, tag="pwb", bufs=2, name="pwb")
                        nc.tensor.matmul(
                            pwb[:], ones128[:],
                            wrow[:, 392 * n2:392 * (n2 + 1)],
                            start=True, stop=True,
                        )
                        nc.scalar.activation(t[:, 392 * n2:392 * (n2 + 1)], pwb[:], AF.Copy)
                    wbc.append(t)

                gat = ctile([128, 4 * L], F32, "gat", bufs=2)
                nc.gpsimd.ap_gather(
                    gat[:], xt_sb[g][:], idxw[g][:],
                    channels=128, num_elems=HW, d=1, num_idxs=4 * L,
                )
                tmp = ctile([128, L], F32, "biltmp", bufs=1)
                nc.vector.tensor_tensor(xs_sb[g][:], gat[:, 0:L], wbc[0][:], op=OP.mult)
                for r in range(1, 4):
                    nc.vector.tensor_tensor(tmp[:], gat[:, L * r:L * (r + 1)],
                                            wbc[r][:], op=OP.mult)
                    nc.vector.tensor_tensor(xs_sb[g][:], xs_sb[g][:], tmp[:], op=OP.add)

        xctx.close()   # release x tiles

        # ---------------- phase H: k and v^T ----------------
        hpool = ctx.enter_context(tc.tile_pool(name="hpool", bufs=1))
        k_sb = [hpool.tile([128, L], BF16, name=f"k_sb{m}") for m in range(3)]
        for m in range(3):
            for n2 in range(2):
                pk = psum.tile([128, 392], F32, tag="big", bufs=2, name="pk")
                for kc in range(3):
                    nc.tensor.matmul(
                        pk[:],
                        kwk_v[kc][:, 128 * m:128 * (m + 1)],
                        xs_sb[kc][:, 392 * n2:392 * (n2 + 1)],
                        start=(kc == 0), stop=(kc == 2),
                    )
                nc.scalar.activation(k_sb[m][:, 392 * n2:392 * (n2 + 1)], pk[:], AF.Copy)

        vTe = [hpool.tile([LC, 6 * 65], BF16, name=f"vTe{lc}") for lc in range(NLC)]
        for lc in range(NLC):
            vv = vTe[lc][:].rearrange("p (h d) -> p h d", h=6)
            nc.vector.memset(vv[:, :, 64:65], 1.0)
            pv = psum.tile([LC, DIM], F32, tag="big", bufs=2, name="pv")
            for kc in range(3):
                nc.tensor.matmul(
                    pv[:],
                    xs_sb[kc][:, LC * lc:LC * (lc + 1)],
                    kwv_v[kc][:, 0:DIM],
                    start=(kc == 0), stop=(kc == 2),
                )
            nc.scalar.activation(vv[:, :, 0:64],
                                 pv[:].rearrange("p (h d) -> p h d", h=6), AF.Copy)

        prectx.close()   # release pre-attention psum

        # ---------------- phase I: attention ----------------
        # rec_all[h, qi*448+j] = 1/denominator for (head h, query qi*448+j)
        rec_all = ctx.enter_context(tc.tile_pool(name="rpool", bufs=1)).tile(
            [NUM_HEAD, HW], F32R, name="rec_all")
        O_all = [qpool.tile([128, HW], BF16, name=f"O_all{m}") for m in range(3)]

        with tc.tile_pool(name="apsum", bufs=1, space="PSUM") as apsum, \
             tc.tile_pool(name="apool", bufs=1) as apool:
            for qi in range(NQC):
                for h in range(NUM_HEAD):
                    m2, hh = h // 2, h % 2
                    psA = apsum.tile([LC, 2, QB], F32, tag="sA", bufs=1, name="psA")
                    psB = apsum.tile([LC, 2, QB], F32, tag="sB", bufs=1, name="psB")
                    psC = apsum.tile([LC, 3, QB], F32, tag="sC", bufs=1, name="psC")
                    ps_o = apsum.tile([65, QC], F32, tag="o", bufs=1, name="ps_o")
                    slot = {0: psA[:, 0, 0:QC], 1: psA[:, 1, 0:QC],
                            2: psB[:, 0, 0:QC], 3: psB[:, 1, 0:QC],
                            4: psC[:, 0, 0:QC], 5: psC[:, 1, 0:QC],
                            6: psC[:, 2, 0:QC]}
                    for lc in range(NLC):
                        nc.tensor.matmul(
                            slot[lc],
                            k_sb[m2][64 * hh:64 * hh + 64, LC * lc:LC * (lc + 1)],
                            q_sb[m2][64 * hh:64 * hh + 64, QC * qi:QC * (qi + 1)],
                            start=True, stop=True,
                        )
                    E_act = apool.tile([LC, 5, QC], BF16, tag="Ea", bufs=2, name="Ea")
                    E_pool = apool.tile([LC, 2, QC], I16, tag="Ep", bufs=2, name="Ep")
                    nc.scalar.activation(E_act[:, 0:2, :], psA[:, :, 0:QC], AF.Exp)
                    nc.vector.tensor_scalar(E_pool[:, :, :], psB[:, :, 0:QC],
                                            SCH_A, SCH_B, OP.mult, OP.add)
                    nc.scalar.activation(E_act[:, 2:5, :], psC[:, :, 0:QC], AF.Exp)

                    rhs = {0: E_act[:, 0, :], 1: E_act[:, 1, :],
                           2: E_pool[:, 0, :].bitcast(BF16),
                           3: E_pool[:, 1, :].bitcast(BF16),
                           4: E_act[:, 2, :], 5: E_act[:, 3, :], 6: E_act[:, 4, :]}
                    for lc in range(NLC):
                        nc.tensor.matmul(
                            ps_o[:],
                            vTe[lc][:, 65 * h:65 * (h + 1)],
                            rhs[lc],
                            start=(lc == 0), stop=(lc == NLC - 1),
                        )
                    nc.vector.tensor_copy(
                        O_all[m2][64 * hh:64 * hh + 64, QC * qi:QC * (qi + 1)],
                        ps_o[0:64, :])
                    rtmp = apool.tile([1, QC], F32R, tag="rtmp", bufs=8, name="rtmp")
                    with nc.allow_low_precision(reason="f32r is fp32-width"):
                        nc.vector.reciprocal(rtmp[:], ps_o[64:65, :])
                    nc.gpsimd.dma_start(
                        rec_all[h:h + 1, QC * qi:QC * (qi + 1)], rtmp[:])

        # ---------------- phase J: normalize + proj ----------------
        with tc.tile_pool(name="ppsum", bufs=1, space="PSUM") as ppsum, \
             tc.tile_pool(name="ppool", bufs=1) as ppool:
            for qi in range(NQC):
                for m in range(3):
                    prb = ppsum.tile([128, QC], F32, tag="rb", bufs=2, name="prb")
                    nc.tensor.matmul(
                        prb[:],
                        ind_sb[:, 128 * m:128 * (m + 1)],
                        rec_all[:, QC * qi:QC * (qi + 1)],
                        start=True, stop=True,
                    )
                    osl = O_all[m][:, QC * qi:QC * (qi + 1)]
                    nc.vector.tensor_tensor(osl, osl, prb[:], op=OP.mult)
                for m in range(3):
                    pp = ppsum.tile([128, QC], F32, tag="pp", bufs=3, name="pp")
                    for kc in range(3):
                        nc.tensor.matmul(
                            pp[:],
                            pjw_v[kc][:, 128 * m:128 * (m + 1)],
                            O_all[kc][:, QC * qi:QC * (qi + 1)],
                            start=(kc == 0), stop=False,
                        )
                    nc.tensor.matmul(
                        pp[:],
                        pjb_sb[:, 128 * m:128 * (m + 1)],
                        ones448[:],
                        start=False, stop=True,
                    )
                    y = ppool.tile([128, QC], F32, tag="y", bufs=3, name="y")
                    nc.scalar.activation(y[:], pp[:], AF.Copy)
                    nc.gpsimd.dma_start(
                        out_d[128 * m:128 * (m + 1), QC * qi:QC * (qi + 1)], y[:])


def host_prep(inputs):
    """Shared (per-core-identical) weight prep. Returns dict of np arrays."""
    f = np.float32
    bf = ml_dtypes.bfloat16
    q_w = np.asarray(inputs["q_w"], f)
    kv_w = np.asarray(inputs["kv_w"], f)
    proj_w = np.asarray(inputs["proj_w"], f)
    proj_b = np.asarray(inputs["proj_b"], f)
    dw_w = np.asarray(inputs["dw_w"], f)
    dw_b = np.asarray(inputs["dw_b"], f)
    bn_w = np.asarray(inputs["bn_w"], f)
    bn_b = np.asarray(inputs["bn_b"], f)
    bn_mean = np.asarray(inputs["bn_mean"], f)
    bn_var = np.asarray(inputs["bn_var"], f)
    pw_w = np.asarray(inputs["pw_w"], f)

    bn_s = (bn_w / np.sqrt(bn_var + BN_EPS)).astype(f)
    bn_t = ((dw_b - bn_mean) * bn_s + bn_b).astype(f)

    p = np.arange(LC)
    c = np.arange(NLC)
    ytab_col = (4 * c[None, :] + p[:, None] // 28 + 0.5 + 2.0 / A).astype(f)  # [112, 7]
    ytab = np.tile(ytab_col, (1, G))                                          # [112, 21]
    xtab_col = (p % 28 + 0.5 + 2.0 / A).astype(f)[:, None]
    xtab = np.tile(xtab_col, (1, G * NLC))

    # block-diagonal dwconv weights: diag[c, 128*t + j] = dw_w[c, t] * (j == c)
    dd = np.zeros((NGD, 25, NGD), f)
    dwf = dw_w.reshape(NGD, 25)
    dd[np.arange(NGD)[:, None], np.arange(25)[None, :], np.arange(NGD)[:, None]] = dwf
    diag = dd.reshape(NGD, 25 * NGD)

    # head-indicator for denominator broadcast: ind6[h, c] = (c // 64 == h)
    ind6 = np.zeros((NUM_HEAD, DIM), f)
    for h in range(NUM_HEAD):
        ind6[h, 64 * h:64 * (h + 1)] = 1.0

    return {
        "qw_t": np.ascontiguousarray(q_w.T).astype(bf),
        "kwk_t": np.ascontiguousarray((kv_w[:DIM] * SCALE).T).astype(bf),
        "kwv_t": np.ascontiguousarray(kv_w[DIM:].T).astype(bf),
        "projw_t": np.ascontiguousarray(proj_w.T).astype(bf),
        "pjb_row": proj_b.reshape(1, DIM).astype(bf),
        "pw_t": np.ascontiguousarray(pw_w.T),
        "diag": diag.astype(bf),
        "ind6": ind6,
        "bn_s": bn_s.reshape(NGD, 1),
        "bn_t": bn_t.reshape(NGD, 1),
        "ytab": ytab,
        "xtab": xtab,
    }


_NC_CACHE = {}


def _get_nc(gelu_exact=True):
    key = bool(gelu_exact)
    if key not in _NC_CACHE:
        _NC_CACHE[key] = build_nc(gelu_exact=key)
    return _NC_CACHE[key]


def make_in_maps(inputs):
    shared = host_prep(inputs)
    x = np.asarray(inputs["x"], np.float32)
    in_maps = []
    for i in range(B):
        m = dict(shared)
        m["x"] = np.ascontiguousarray(x[i].reshape(DIM, HW))
        in_maps.append(m)
    return in_maps


def run_spmd(inputs, trace=False):
    """Run on the 8 NeuronCores; returns (out (8,384,56,56), BassKernelResults)."""
    nc = _get_nc(True)
    in_maps = make_in_maps(inputs)
    res = bass_utils.run_bass_kernel_spmd(
        nc, in_maps, core_ids=list(range(B)), trace=trace,
    )
    out = np.stack([r["out"].reshape(DIM, H, W) for r in res.results], axis=0)
    return out, res


def kernel(**inputs) -> np.ndarray:
    out, _ = run_spmd(inputs, trace=False)
    return out
